# revision 21
# baseline (speedup 1.0000x reference)
"""Trainium2 Bass kernel for nn_Clusterator (soft k-means / vq_codebook).

reference:
    mu_init, _ = cluster(embeds, init, temp=30, num_iter=10)
    mu, r     = cluster(embeds, mu_init, temp=30, num_iter=1)
which is exactly 11 iterations of:
    mun  = mu / (||mu||_row + 1e-6)
    dist = data_n @ mun.T          # data_n = row-normalized embeds
    r    = softmax(30 * dist, axis=1)
    mu   = (r.T @ data_n) / r.sum(0)[:, None]
returning mu from iteration 11 and r = softmax(30 * dist_11).

Sharding: data-parallel over N=200k rows on 8 cores (25088 rows/core after
padding to 200704).  Each core keeps its shard SBUF-resident in two layouts:
xT f32 [128(d), 25088(n)] (stationary weights for dist matmuls) and
x_aug bf16 [128, 196, 129] ([row, 1-flag], rhs for the cluster-mean matmuls;
pad rows have an all-zero x_aug row so they contribute nothing).  Per
iteration the [64,129] partial [cluster_mean | cluster_r] is AllGathered
across the 8 cores and reduced locally.
"""

import numpy as np

import concourse.bacc as bacc
import concourse.bass as bass
import concourse.tile as tile
from concourse import mybir
from concourse.bass_utils import run_bass_kernel_spmd

F32 = mybir.dt.float32
BF16 = mybir.dt.bfloat16
AF = mybir.ActivationFunctionType
ALU = mybir.AluOpType

N_FULL = 200000
D = 128
K = 64
N_CORES = 8
TILES = 196            # tiles of 128 rows per core
ROWS = TILES * 128     # 25088 ; 8*25088 = 200704 >= 200000
GROUP = 7              # tiles per pipeline group
TEMP = 30.0
EPS = 1e-6
N_ITER = 11


def _inv_norm_newton(nc, inv_out, ss, y0, tmp, eps, bias_ap):
    """inv_out = 1/(sqrt(ss) + eps), with one Newton step to clean up the
    ACT sqrt table (which has a loose precision budget).
    ss/y0/tmp/inv_out: same-shape f32 APs ([P, n]).
    bias_ap: [P, 1] SBUF f32 tile holding 1e-30 — keeps the Newton 1/y0
    finite for all-zero (pad) rows while negligible for real rows."""
    p = ss.shape[0]
    nc.scalar.activation(y0, ss, AF.Sqrt, bias=bias_ap[0:p, 0:1])
    nc.vector.reciprocal(tmp, y0)                      # 1/y0
    nc.vector.tensor_mul(tmp, ss, tmp)                 # ss/y0
    nc.vector.tensor_add(tmp, tmp, y0)                 # y0 + ss/y0
    nc.vector.tensor_scalar(tmp, tmp, 0.5, eps, ALU.mult, ALU.add)
    # tmp = 0.5*(y0 + ss/y0) + eps  ==  sqrt(ss) + eps
    nc.vector.reciprocal(inv_out, tmp)


def build_cluster_kernel(tc, outs, ins, cores=N_CORES, tiles=TILES,
                         group=GROUP, n_iter=N_ITER, temp=TEMP):
    nc = tc.nc
    x_in = ins["x_in"]      # [tiles*128, 129] f32 (col 128: 1.0 valid / 0.0 pad)
    init = ins["init"]      # [K, D] f32
    ident = ins["ident"]    # [128, 128] f32 identity
    fold = ins["fold64"]    # [128, 64] f32 = [I64; I64] partition-fold matrix
    r_out = outs["r_out"]   # [tiles*128, K] f32
    mu_out = outs["mu_out"]  # [K, D] f32

    n_groups = tiles // group
    assert n_groups * group == tiles
    gcols = group * 128     # data columns per group
    gk = group * K          # dist/e/r columns per group

    x_in_g = x_in.rearrange("(g t p) f -> g p t f", p=128, t=group)
    r_out_g = r_out.rearrange("(g t p) k -> g p t k", p=128, t=group)

    with tc.tile_pool(name="persist", bufs=1) as persist:
        xT = persist.tile([128, tiles * 128], F32, name="xT")
        x_aug = persist.tile([128, tiles, 129], BF16, name="x_aug")
        ident_sb = persist.tile([128, 128], F32, name="ident_sb")
        fold_sb = persist.tile([128, K], F32, name="fold_sb")
        mu_sb = persist.tile([K, D], F32, name="mu_sb")

        sqbias = persist.tile([128, 1], F32, name="sqbias")
        nc.vector.memset(sqbias[:], 1e-30)
        nc.sync.dma_start(ident_sb[:], ident[:])
        nc.sync.dma_start(fold_sb[:], fold[:])
        nc.sync.dma_start(mu_sb[:], init[:])

        # ---------------- startup: normalize rows, build xT + x_aug ------
        with tc.tile_pool(name="ld", bufs=3) as ld, \
             tc.tile_pool(name="ldw", bufs=3) as ldw, \
             tc.tile_pool(name="ldps", bufs=4, space="PSUM") as ldps:
            for g in range(n_groups):
                xin_g = ld.tile([128, group, 129], F32, name="xin_g")
                nc.sync.dma_start(xin_g[:], x_in_g[g])
                xd = xin_g[:, :, 0:128]
                # row sums of squares (per tile column j)
                sq = ldw.tile([128, gcols], F32, name="sq")
                nc.scalar.activation(sq[:], xd, AF.Square)
                ss = ldw.tile([128, group], F32, name="ss")
                nc.vector.reduce_sum(ss[:], sq.rearrange("p (t f) -> p t f", t=group),
                                     axis=mybir.AxisListType.X)
                invn = ldw.tile([128, group], F32, name="invn")
                y0 = ldw.tile([128, group], F32, name="y0")
                tmpn = ldw.tile([128, group], F32, name="tmpn")
                _inv_norm_newton(nc, invn[:], ss[:], y0[:], tmpn[:], EPS, sqbias)
                # normalized rows (f32)
                xn = ldw.tile([128, group, 128], F32, name="xn")
                nc.vector.tensor_tensor(
                    xn[:], xd, invn.broadcast_to([128, group, 128]), ALU.mult)
                # bf16 copy into x_aug data cols + validity flag col
                nc.vector.tensor_copy(x_aug[:, g * group:(g + 1) * group, 0:128],
                                      xn[:])
                nc.vector.tensor_copy(x_aug[:, g * group:(g + 1) * group, 128:129],
                                      xin_g[:, :, 128:129])
                # transposes into xT
                for j in range(group):
                    t = g * group + j
                    xt_ps = ldps.tile([128, 128], F32, name="xt_ps")
                    nc.tensor.transpose(xt_ps[:], xn[:, j, :], ident_sb[:])
                    nc.scalar.copy(xT[:, t * 128:(t + 1) * 128], xt_ps[:])

        # ---------------- iterations ------------------------------------
        with tc.tile_pool(name="wk", bufs=3) as wk, \
             tc.tile_pool(name="sbuf_s", bufs=2) as sbuf_s, \
             tc.tile_pool(name="mupool", bufs=2) as mupool, \
             tc.tile_pool(name="ps", bufs=3, space="PSUM") as ps, \
             tc.tile_pool(name="ps_cm", bufs=1, space="PSUM") as ps_cm, \
             tc.tile_pool(name="ps_t", bufs=1, space="PSUM") as ps_t, \
             tc.tile_pool(name="dram", bufs=2, space="DRAM") as dram:
            for it in range(n_iter):
                last = (it == n_iter - 1)
                # --- mun = mu / (||mu|| + eps), transposed to [D, K] -----
                musq = mupool.tile([K, D], F32, name="musq")
                mss = mupool.tile([K, 4], F32, name="mss")
                nc.scalar.activation(musq[:], mu_sb[:], AF.Square,
                                     accum_out=mss[:, 0:1])
                _inv_norm_newton(nc, mss[:, 3:4], mss[:, 0:1], mss[:, 1:2],
                                 mss[:, 2:3], EPS, sqbias)
                mun = mupool.tile([K, D], F32, name="mun")
                nc.vector.tensor_scalar(mun[:], mu_sb[:], mss[:, 3:4], None,
                                        ALU.mult)
                munT_ps = ps_t.tile([128, K], F32, name="munT_ps")
                nc.tensor.transpose(munT_ps[:], mun[:], ident_sb[0:K, 0:K])
                munT = mupool.tile([128, K], F32, name="munT")
                nc.scalar.copy(munT[:], munT_ps[:])

                s_buf = sbuf_s.tile([128, tiles], F32, name="s_buf")
                inv_s = sbuf_s.tile([128, tiles], F32, name="inv_s")
                # two separate PSUM banks: start=True clears has_written for a
                # whole bank, so the two interleaved accumulation chains
                # (PE col-groups 0:64 / 64:128) must not share one.
                cm_a = ps_cm.tile([128, 129], F32, name="cm_a", tag="cm_a")
                cm_b = ps_cm.tile([128, 129], F32, name="cm_b", tag="cm_b")

                for g in range(n_groups):
                    dist_ps = ps.tile([128, gk], F32, name="dist_ps")
                    for j in range(group):
                        t = g * group + j
                        nc.tensor.matmul(dist_ps[:, j * K:(j + 1) * K],
                                         xT[:, t * 128:(t + 1) * 128],
                                         munT[:], start=True, stop=True)
                    edt = F32 if last else BF16
                    e_g = wk.tile([128, gk], F32 if last else BF16, name="e_g",
                                  tag="e_g32" if last else "e_g16")
                    nc.scalar.activation(e_g[:], dist_ps[:], AF.Exp, scale=temp)
                    sl = slice(g * group, (g + 1) * group)
                    nc.vector.reduce_sum(
                        s_buf[:, sl],
                        e_g.rearrange("p (t k) -> p t k", t=group),
                        axis=mybir.AxisListType.X)
                    nc.vector.reciprocal(inv_s[:, sl], s_buf[:, sl])
                    inv_b = inv_s[:, sl].broadcast_to([128, group, K])
                    r_g = wk.tile([128, group, K], BF16, name="r_g")
                    nc.vector.tensor_tensor(
                        r_g[:], e_g.rearrange("p (t k) -> p t k", t=group),
                        inv_b, ALU.mult)
                    if last:
                        r32_g = wk.tile([128, group, K], F32, name="r32_g")
                        nc.vector.tensor_tensor(
                            r32_g[:], e_g.rearrange("p (t k) -> p t k", t=group),
                            inv_b, ALU.mult)
                        nc.sync.dma_start(r_out_g[g], r32_g[:])
                    for j in range(group):
                        t = g * group + j
                        cm_t = cm_a[0:K, :] if t % 2 == 0 else cm_b[K:128, :]
                        nc.tensor.matmul(
                            cm_t, r_g[:, j, :], x_aug[:, t, :],
                            start=(t < 2), stop=(t >= tiles - 2))

                # --- both PSUM halves -> one aligned SBUF tile -----------
                # (engines cannot shift partitions; the 0:64 / 64:128 halves
                # stay in place and are folded by a matmul against
                # fold = [I64; I64] later.)
                cm_sb = mupool.tile([128, 129], F32, name="cm_sb")
                nc.scalar.copy(cm_sb[0:K, :], cm_a[0:K, :])
                nc.scalar.copy(cm_sb[K:128, :], cm_b[K:128, :])

                # --- AllGather [128,129] per rank + local reduce ---------
                if cores > 1:
                    ag_in = dram.tile([128, 129], F32, name="ag_in")
                    ag_out = dram.tile([128 * cores, 129], F32, name="ag_out",
                                       addr_space="Shared" if cores > 4 else "Local")
                    nc.sync.dma_start(ag_in[:], cm_sb[:])
                    nc.gpsimd.collective_compute(
                        "AllGather", ALU.bypass,
                        replica_groups=[list(range(cores))],
                        ins=[ag_in.opt()], outs=[ag_out.opt()])
                    gath = mupool.tile([128, cores, 129], F32, name="gath")
                    nc.sync.dma_start(
                        gath[:], ag_out.rearrange("(a p) f -> p a f", p=128))
                    # tree-sum the rank slices (partition-aligned)
                    w = cores
                    cur = gath
                    while w > 1:
                        w //= 2
                        nxt = mupool.tile([128, w, 129], F32,
                                          name=f"red{w}", tag=f"red{w}")
                        nc.vector.tensor_add(nxt[:], cur[:, 0:w, :],
                                             cur[:, w:2 * w, :])
                        cur = nxt
                    total = cur.rearrange("p a f -> p (a f)")
                else:
                    total = cm_sb

                # --- stats[k,:] = total[k,:] + total[k+64,:] via PE ------
                stats_ps = ps_t.tile([K, 129], F32, name="stats_ps",
                                     tag="stats_ps")
                nc.tensor.matmul(stats_ps[:], fold_sb[:], total,
                                 start=True, stop=True)

                # --- mu = cluster_mean / cluster_r -----------------------
                crinv = mupool.tile([K, 1], F32, name="crinv")
                nc.vector.reciprocal(crinv[:], stats_ps[:, 128:129])
                nc.vector.tensor_scalar(mu_sb[:], stats_ps[:, 0:128], crinv[:],
                                        None, ALU.mult)
                if last:
                    nc.sync.dma_start(mu_out[:], mu_sb[:])


# ----------------------------------------------------------------------------
# host wrapper
# ----------------------------------------------------------------------------
_CACHED = {}


def _build_hw():
    if "nc" in _CACHED:
        return _CACHED["nc"]
    nc = bacc.Bacc("TRN2", target_bir_lowering=False, debug=False,
                   enable_asserts=False, num_devices=N_CORES)
    ins = {
        "x_in": nc.dram_tensor("x_in", [ROWS, 129], F32,
                               kind="ExternalInput").ap(),
        "init": nc.dram_tensor("init", [K, D], F32,
                               kind="ExternalInput").ap(),
        "ident": nc.dram_tensor("ident", [128, 128], F32,
                                kind="ExternalInput").ap(),
        "fold64": nc.dram_tensor("fold64", [128, K], F32,
                                 kind="ExternalInput").ap(),
    }
    outs = {
        "r_out": nc.dram_tensor("r_out", [ROWS, K], F32,
                                kind="ExternalOutput").ap(),
        "mu_out": nc.dram_tensor("mu_out", [K, D], F32,
                                 kind="ExternalOutput").ap(),
    }
    with tile.TileContext(nc) as tc:
        build_cluster_kernel(tc, outs, ins)
    nc.compile()
    _CACHED["nc"] = nc
    return nc


def _prep_in_maps(embeds, init):
    n = embeds.shape[0]
    x = np.zeros((ROWS * N_CORES, 129), dtype=np.float32)
    x[:n, 0:128] = embeds
    x[:n, 128] = 1.0
    ident = np.eye(128, dtype=np.float32)
    fold64 = np.vstack([np.eye(K, dtype=np.float32),
                        np.eye(K, dtype=np.float32)])
    init = np.ascontiguousarray(np.asarray(init, np.float32))
    return [
        {"x_in": np.ascontiguousarray(x[c * ROWS:(c + 1) * ROWS]),
         "init": init, "ident": ident, "fold64": fold64}
        for c in range(N_CORES)
    ]


def run_hw(embeds, init, trace=False):
    nc = _build_hw()
    in_maps = _prep_in_maps(embeds, init)
    res = run_bass_kernel_spmd(nc, in_maps,
                               core_ids=list(range(N_CORES)), trace=trace)
    r = np.concatenate([res.results[c]["r_out"] for c in range(N_CORES)],
                       axis=0)[:N_FULL]
    mu = res.results[0]["mu_out"]
    return (mu, r), res


def kernel(embeds, init, cluster_temp):
    assert int(np.asarray(cluster_temp)) == 30
    (mu, r), _ = run_hw(np.asarray(embeds, np.float32),
                        np.asarray(init, np.float32))
    return mu, r


# revision 27
# speedup vs baseline: 1.8164x; 1.8164x over previous
"""Trainium2 Bass kernel for nn_Clusterator (soft k-means / vq_codebook).

reference:
    mu_init, _ = cluster(embeds, init, temp=30, num_iter=10)
    mu, r     = cluster(embeds, mu_init, temp=30, num_iter=1)
which is exactly 11 iterations of:
    mun  = mu / (||mu||_row + 1e-6)
    dist = data_n @ mun.T          # data_n = row-normalized embeds
    r    = softmax(30 * dist, axis=1)
    mu   = (r.T @ data_n) / r.sum(0)[:, None]
returning mu from iteration 11 and r = softmax(30 * dist_11).

Sharding: data-parallel over N=200k rows on 8 cores (25088 rows/core after
zero-padding to 200704).  Each core keeps its shard SBUF-resident:
  xT_hi/xT_lo  bf16 [128(d), 25088(n)]  hi/lo split of normalized rows,
               transposed — stationary weights for the dist matmuls
               (dist = xh@mh + xh@ml + xl@mh keeps ~f32 accuracy while
               every matmul stays bf16, which also keeps the PE HAM
               clock-gate warm: f32 matmuls don't count as PE activity).
  x_aug        bf16 [128, 196, 129] = [normalized row (hi), valid-flag] —
               rhs of the cluster-mean matmuls; pad rows are all-zero so
               they contribute nothing to the cluster sums.
Cluster-mean matmuls run in pairs: lhsT = [r_t | r_t+1] (one 128-col FWL
weight load), rhs = [x_t | x_t+1] (N=258); the two good quadrants are
picked out afterwards by fold matmuls against identity slices.
Per iteration the [64,129] partial [cluster_mean | cluster_r] is
AllGathered across the 8 cores and reduced locally.  For iterations 0..9
mu is never materialized: row_normalize(cm/cr) == cm/||cm|| since cr>0.
"""

import numpy as np

import concourse.bacc as bacc
import concourse.bass as bass
import concourse.tile as tile
from concourse import mybir
from concourse.bass_utils import run_bass_kernel_spmd

F32 = mybir.dt.float32
BF16 = mybir.dt.bfloat16
AF = mybir.ActivationFunctionType
ALU = mybir.AluOpType

N_FULL = 200000
D = 128
K = 64
N_CORES = 8
TILES = 196            # tiles of 128 rows per core
ROWS = TILES * 128     # 25088 ; 8*25088 = 200704 >= 200000
GROUP = 14             # tiles per pipeline group (even: B-pair matmuls)
TEMP = 30.0
EPS = 1e-6
N_ITER = 11


def _rsqrt_ln_exp(nc, pool, ss, p, n, sqbias, newton=1, name="rs"):
    """y ~= 1/sqrt(ss) via exp(-0.5*ln(ss + 1e-30)) + Newton refinement.
    Uses only Ln/Exp (same ACT table set as the softmax Exp -> no table
    switches) and stays finite for all-zero (pad) rows.
    Returns a [p, n] f32 AP."""
    lg = pool.tile([p, n], F32, name=f"{name}_lg", tag=f"{name}_lg")
    y = pool.tile([p, n], F32, name=f"{name}_y", tag=f"{name}_y")
    t1 = pool.tile([p, n], F32, name=f"{name}_t1", tag=f"{name}_t1")
    nc.scalar.activation(lg[:], ss, AF.Ln, bias=sqbias[0:p, 0:1])
    nc.scalar.activation(y[:], lg[:], AF.Exp, scale=-0.5)
    for _ in range(newton):
        # y <- y * (1.5 - 0.5 * ss * y^2)
        nc.vector.tensor_mul(t1[:], y[:], y[:])
        nc.vector.tensor_mul(t1[:], ss, t1[:])
        nc.vector.tensor_scalar(t1[:], t1[:], -0.5, 1.5, ALU.mult, ALU.add)
        nc.vector.tensor_mul(y[:], y[:], t1[:])
    return y


def build_cluster_kernel(tc, outs, ins, cores=N_CORES, tiles=TILES,
                         group=GROUP, n_iter=N_ITER, temp=TEMP):
    nc = tc.nc
    x_in = ins["x_in"]      # [tiles*128, 129] f32 (col 128: 1.0 valid / 0.0 pad)
    init = ins["init"]      # [K, D] f32
    ident = ins["ident"]    # [128, 128] f32 identity
    identb = ins["identb"]  # [128, 128] bf16 identity (transpose of bf16 tiles)
    fold = ins["fold64"]    # [128, 64] f32 = [I64; I64] partition-fold matrix
    r_out = outs["r_out"]   # [tiles*128, K] f32
    mu_out = outs["mu_out"]  # [K, D] f32

    n_groups = tiles // group
    assert n_groups * group == tiles and group % 2 == 0
    gk = group * K          # dist/e/r columns per group

    x_in_g = x_in.rearrange("(g t p) f -> g p t f", p=128, t=group)
    r_out_g = r_out.rearrange("(g t p) k -> g p t k", p=128, t=group)

    with tc.tile_pool(name="persist", bufs=1) as persist:
        xT_hi = persist.tile([128, tiles * 128], BF16, name="xT_hi")
        xT_lo = persist.tile([128, tiles * 128], BF16, name="xT_lo")
        x_aug = persist.tile([128, tiles, 129], BF16, name="x_aug")
        ident_sb = persist.tile([128, 128], F32, name="ident_sb")
        identb_sb = persist.tile([128, 128], BF16, name="identb_sb")
        fold_sb = persist.tile([128, K], F32, name="fold_sb")
        mu_sb = persist.tile([K, D], F32, name="mu_sb")
        sqbias = persist.tile([128, 1], F32, name="sqbias")

        nc.vector.memset(sqbias[:], 1e-30)
        nc.sync.dma_start(ident_sb[:], ident[:])
        nc.sync.dma_start(identb_sb[:], identb[:])
        nc.sync.dma_start(fold_sb[:], fold[:])
        nc.sync.dma_start(mu_sb[:], init[:])

        # ---------------- startup: normalize rows, build splits ----------
        with tc.tile_pool(name="ld", bufs=2) as ld, \
             tc.tile_pool(name="ldw", bufs=2) as ldw, \
             tc.tile_pool(name="ldps", bufs=3, space="PSUM") as ldps:
            for g in range(n_groups):
                xin_g = ld.tile([128, group, 129], F32, name="xin_g")
                nc.sync.dma_start(xin_g[:], x_in_g[g])
                xd = xin_g[:, :, 0:128]
                # xn doubles as the Square scratch before being overwritten
                # with the normalized rows (saves SBUF).
                xn = ldw.tile([128, group, 128], F32, name="xn")
                nc.scalar.activation(xn[:], xd, AF.Square)
                ss = ldw.tile([128, group], F32, name="ss")
                nc.vector.reduce_sum(ss[:], xn[:],
                                     axis=mybir.AxisListType.X)
                # inv = 1/(sqrt(ss)+eps): rsqrt then norm=ss*y, +eps, recip
                y = _rsqrt_ln_exp(nc, ldw, ss[:], 128, group, sqbias,
                                  newton=1, name="ld")
                nrm = ldw.tile([128, group], F32, name="nrm")
                nc.vector.tensor_mul(nrm[:], ss[:], y[:])
                nc.vector.tensor_scalar(nrm[:], nrm[:], 1.0, EPS,
                                        ALU.mult, ALU.add)
                invn = ldw.tile([128, group], F32, name="invn")
                nc.vector.reciprocal(invn[:], nrm[:])
                nc.vector.tensor_tensor(
                    xn[:], xd, invn.broadcast_to([128, group, 128]), ALU.mult)
                # x_aug: bf16(xn) data cols + validity flag col
                sl = slice(g * group, (g + 1) * group)
                nc.vector.tensor_copy(x_aug[:, sl, 0:128], xn[:])
                nc.vector.tensor_copy(x_aug[:, sl, 128:129],
                                      xin_g[:, :, 128:129])
                # lo residual (bf16): xn - bf16(xn)
                xl_g = ldw.tile([128, group, 128], BF16, name="xl_g")
                nc.vector.tensor_tensor(xl_g[:], xn[:],
                                        x_aug[:, sl, 0:128], ALU.subtract)
                # transposes into xT_hi / xT_lo
                for j in range(group):
                    t = g * group + j
                    th = ldps.tile([128, 128], BF16, name="th", tag="th")
                    nc.tensor.transpose(th[:], x_aug[:, t, 0:128],
                                        identb_sb[:])
                    nc.scalar.copy(xT_hi[:, t * 128:(t + 1) * 128], th[:])
                    tl = ldps.tile([128, 128], BF16, name="tl", tag="tl")
                    nc.tensor.transpose(tl[:], xl_g[:, j, :], identb_sb[:])
                    nc.scalar.copy(xT_lo[:, t * 128:(t + 1) * 128], tl[:])

        # ---------------- iterations ------------------------------------
        with tc.tile_pool(name="wk", bufs=3) as wk, \
             tc.tile_pool(name="sbuf_s", bufs=2) as sbuf_s, \
             tc.tile_pool(name="mupool", bufs=2) as mupool, \
             tc.tile_pool(name="ps", bufs=2, space="PSUM") as ps, \
             tc.tile_pool(name="ps_cm", bufs=1, space="PSUM") as ps_cm, \
             tc.tile_pool(name="ps_t", bufs=1, space="PSUM") as ps_t, \
             tc.tile_pool(name="dram", bufs=2, space="DRAM") as dram:
            for it in range(n_iter):
                last = (it == n_iter - 1)
                # --- mun (normalized mu), bf16 hi/lo, transposed ---------
                # it==0: mun = init/||init||; else mun = cm/||cm|| (== the
                # row-normalized cm/cr since cr > 0; eps effect ~1e-6).
                src = mu_sb[:] if it == 0 else stats_ps[:, 0:128]
                musq = mupool.tile([K, D], F32, name="musq")
                mss = mupool.tile([K, 1], F32, name="mss")
                nc.scalar.activation(musq[:], src, AF.Square,
                                     accum_out=mss[:, 0:1])
                ymu = _rsqrt_ln_exp(nc, mupool, mss[:, 0:1], K, 1, sqbias,
                                    newton=2, name="mu")
                mun = mupool.tile([K, D], F32, name="mun")
                nc.vector.tensor_scalar(mun[:], src, ymu[:], None, ALU.mult)
                mh = mupool.tile([K, D], BF16, name="mh")
                nc.vector.tensor_copy(mh[:], mun[:])
                ml = mupool.tile([K, D], BF16, name="ml")
                nc.vector.tensor_tensor(ml[:], mun[:], mh[:], ALU.subtract)
                mhT = mupool.tile([128, K], BF16, name="mhT")
                mlT = mupool.tile([128, K], BF16, name="mlT")
                for msrc, mdst in ((mh, mhT), (ml, mlT)):
                    tp = ps_t.tile([128, K], BF16, name="tp", tag="tp")
                    nc.tensor.transpose(tp[:], msrc[:], identb_sb[0:K, 0:K])
                    nc.scalar.copy(mdst[:], tp[:])

                s_buf = sbuf_s.tile([128, tiles], F32, name="s_buf")
                inv_s = sbuf_s.tile([128, tiles], F32, name="inv_s")
                cm_ps = ps_cm.tile([128, 258], F32, name="cm_ps")

                for g in range(n_groups):
                    dist_ps = ps.tile([128, gk], F32, name="dist_ps")
                    for j in range(group):
                        t = g * group + j
                        dsl = dist_ps[:, j * K:(j + 1) * K]
                        xh_t = xT_hi[:, t * 128:(t + 1) * 128]
                        xl_t = xT_lo[:, t * 128:(t + 1) * 128]
                        nc.tensor.matmul(dsl, xh_t, mhT[:],
                                         start=True, stop=False)
                        nc.tensor.matmul(dsl, xh_t, mlT[:],
                                         start=False, stop=False)
                        nc.tensor.matmul(dsl, xl_t, mhT[:],
                                         start=False, stop=True)
                    e_g = wk.tile([128, gk], F32 if last else BF16, name="e_g",
                                  tag="e_g32" if last else "e_g16",
                                  bufs=2 if last else None)
                    nc.scalar.activation(e_g[:], dist_ps[:], AF.Exp, scale=temp)
                    sl = slice(g * group, (g + 1) * group)
                    nc.vector.reduce_sum(
                        s_buf[:, sl],
                        e_g.rearrange("p (t k) -> p t k", t=group),
                        axis=mybir.AxisListType.X)
                    nc.vector.reciprocal(inv_s[:, sl], s_buf[:, sl])
                    inv_b = inv_s[:, sl].broadcast_to([128, group, K])
                    r_g = wk.tile([128, group, K], BF16, name="r_g")
                    nc.vector.tensor_tensor(
                        r_g[:], e_g.rearrange("p (t k) -> p t k", t=group),
                        inv_b, ALU.mult)
                    if last:
                        r32_g = wk.tile([128, group, K], F32, name="r32_g",
                                        bufs=2)
                        nc.vector.tensor_tensor(
                            r32_g[:], e_g.rearrange("p (t k) -> p t k", t=group),
                            inv_b, ALU.mult)
                        nc.sync.dma_start(r_out_g[g], r32_g[:])
                    # cluster-mean matmuls in pairs: one 128-col weight
                    # load for two tiles; good quadrants: rows 0:64 x
                    # cols 0:129 (tile t) and rows 64:128 x cols 129:258
                    # (tile t+1).
                    for j in range(0, group, 2):
                        t = g * group + j
                        nc.tensor.matmul(
                            cm_ps[:, :],
                            r_g[:, j:j + 2, :],
                            x_aug[:, t:t + 2, :],
                            start=(t == 0), stop=(t == tiles - 2))

                # --- pre-fold good quadrants -> partial [K, 129] ---------
                cm_sb = mupool.tile([128, 258], F32, name="cm_sb")
                nc.scalar.copy(cm_sb[:], cm_ps[:])
                partial_ps = ps_t.tile([K, 129], F32, name="partial_ps",
                                       tag="foldout")
                nc.tensor.matmul(partial_ps[:], ident_sb[:, 0:K],
                                 cm_sb[:, 0:129], start=True, stop=False)
                nc.tensor.matmul(partial_ps[:], ident_sb[:, K:128],
                                 cm_sb[:, 129:258], start=False, stop=True)

                # --- AllGather + local reduce ----------------------------
                if cores > 1:
                    partial_sb = mupool.tile([K, 129], F32, name="partial_sb")
                    nc.scalar.copy(partial_sb[:], partial_ps[:])
                    ag_in = dram.tile([K, 129], F32, name="ag_in")
                    ag_out = dram.tile([K * cores, 129], F32, name="ag_out",
                                       addr_space="Shared" if cores > 4 else "Local")
                    nc.sync.dma_start(ag_in[:], partial_sb[:])
                    nc.gpsimd.collective_compute(
                        "AllGather", ALU.bypass,
                        replica_groups=[list(range(cores))],
                        ins=[ag_in.opt()], outs=[ag_out.opt()])
                    pairs = (K * cores) // 128
                    gath = mupool.tile([128, max(pairs, 1), 129], F32,
                                       name="gath")
                    nc.sync.dma_start(
                        gath[:], ag_out.rearrange("(a p) f -> p a f", p=128))
                    w = pairs
                    cur = gath
                    while w > 1:
                        w //= 2
                        nxt = mupool.tile([128, w, 129], F32,
                                          name=f"red{w}", tag=f"red{w}")
                        nc.vector.tensor_add(nxt[:], cur[:, 0:w, :],
                                             cur[:, w:2 * w, :])
                        cur = nxt
                    total = cur.rearrange("p a f -> p (a f)")
                    # stats[k,:] = total[k,:] + total[k+64,:] via fold mm
                    stats_ps = ps_t.tile([K, 129], F32, name="stats_ps",
                                         tag="foldout")
                    nc.tensor.matmul(stats_ps[:], fold_sb[:], total,
                                     start=True, stop=True)
                else:
                    stats_ps = partial_ps

                if last:
                    # mu = cluster_mean / cluster_r (not normalized)
                    crinv = mupool.tile([K, 1], F32, name="crinv")
                    nc.vector.reciprocal(crinv[:], stats_ps[:, 128:129])
                    mu_fin = mupool.tile([K, D], F32, name="mu_fin")
                    nc.vector.tensor_scalar(mu_fin[:], stats_ps[:, 0:128],
                                            crinv[:], None, ALU.mult)
                    nc.sync.dma_start(mu_out[:], mu_fin[:])


# ----------------------------------------------------------------------------
# host wrapper
# ----------------------------------------------------------------------------
_CACHED = {}


def _build_hw():
    if "nc" in _CACHED:
        return _CACHED["nc"]
    nc = bacc.Bacc("TRN2", target_bir_lowering=False, debug=False,
                   enable_asserts=False, num_devices=N_CORES)
    ins = {
        "x_in": nc.dram_tensor("x_in", [ROWS, 129], F32,
                               kind="ExternalInput").ap(),
        "init": nc.dram_tensor("init", [K, D], F32,
                               kind="ExternalInput").ap(),
        "ident": nc.dram_tensor("ident", [128, 128], F32,
                                kind="ExternalInput").ap(),
        "identb": nc.dram_tensor("identb", [128, 128], BF16,
                                 kind="ExternalInput").ap(),
        "fold64": nc.dram_tensor("fold64", [128, K], F32,
                                 kind="ExternalInput").ap(),
    }
    outs = {
        "r_out": nc.dram_tensor("r_out", [ROWS, K], F32,
                                kind="ExternalOutput").ap(),
        "mu_out": nc.dram_tensor("mu_out", [K, D], F32,
                                 kind="ExternalOutput").ap(),
    }
    with tile.TileContext(nc) as tc:
        build_cluster_kernel(tc, outs, ins)
    nc.compile()
    _CACHED["nc"] = nc
    return nc


def _prep_in_maps(embeds, init):
    n = embeds.shape[0]
    x = np.zeros((ROWS * N_CORES, 129), dtype=np.float32)
    x[:n, 0:128] = embeds
    x[:n, 128] = 1.0
    import ml_dtypes
    ident = np.eye(128, dtype=np.float32)
    identb = np.eye(128, dtype=np.float32).astype(ml_dtypes.bfloat16)
    fold64 = np.vstack([np.eye(K, dtype=np.float32),
                        np.eye(K, dtype=np.float32)])
    init = np.ascontiguousarray(np.asarray(init, np.float32))
    return [
        {"x_in": np.ascontiguousarray(x[c * ROWS:(c + 1) * ROWS]),
         "init": init, "ident": ident, "identb": identb, "fold64": fold64}
        for c in range(N_CORES)
    ]


def run_hw(embeds, init, trace=False):
    nc = _build_hw()
    in_maps = _prep_in_maps(embeds, init)
    res = run_bass_kernel_spmd(nc, in_maps,
                               core_ids=list(range(N_CORES)), trace=trace)
    r = np.concatenate([res.results[c]["r_out"] for c in range(N_CORES)],
                       axis=0)[:N_FULL]
    mu = res.results[0]["mu_out"]
    return (mu, r), res


def kernel(embeds, init, cluster_temp):
    assert int(np.asarray(cluster_temp)) == 30
    (mu, r), _ = run_hw(np.asarray(embeds, np.float32),
                        np.asarray(init, np.float32))
    return mu, r


# revision 32
# speedup vs baseline: 1.9243x; 1.0594x over previous
"""Trainium2 Bass kernel for nn_Clusterator (soft k-means / vq_codebook).

reference:
    mu_init, _ = cluster(embeds, init, temp=30, num_iter=10)
    mu, r     = cluster(embeds, mu_init, temp=30, num_iter=1)
which is exactly 11 iterations of:
    mun  = mu / (||mu||_row + 1e-6)
    dist = data_n @ mun.T          # data_n = row-normalized embeds
    r    = softmax(30 * dist, axis=1)
    mu   = (r.T @ data_n) / r.sum(0)[:, None]
returning mu from iteration 11 and r = softmax(30 * dist_11).

Sharding: data-parallel over N=200k rows on 8 cores (25088 rows/core after
zero-padding to 200704).  Each core keeps its shard SBUF-resident:
  xT_hi/xT_lo  bf16 [128(d), 25088(n)]  hi/lo split of normalized rows,
               transposed — stationary weights for the dist matmuls
               (dist = xh@mh + xh@ml + xl@mh keeps ~f32 accuracy while
               every matmul stays bf16, which also keeps the PE HAM
               clock-gate warm: f32 matmuls don't count as PE activity).
  x_aug        bf16 [128, 196, 129] = [normalized row (hi), valid-flag] —
               rhs of the cluster-mean matmuls; pad rows are all-zero so
               they contribute nothing to the cluster sums.
Cluster-mean matmuls run in pairs: lhsT = [r_t | r_t+1] (one 128-col FWL
weight load), rhs = [x_t | x_t+1] (N=258); the two good quadrants are
picked out afterwards by fold matmuls against identity slices.
Per iteration the [64,129] partial [cluster_mean | cluster_r] is
AllGathered across the 8 cores and reduced locally.  For iterations 0..9
mu is never materialized: row_normalize(cm/cr) == cm/||cm|| since cr>0.
"""

import numpy as np

import concourse.bacc as bacc
import concourse.bass as bass
import concourse.tile as tile
from concourse import mybir
from concourse.bass_utils import run_bass_kernel_spmd

F32 = mybir.dt.float32
BF16 = mybir.dt.bfloat16
AF = mybir.ActivationFunctionType
ALU = mybir.AluOpType

N_FULL = 200000
D = 128
K = 64
N_CORES = 8
TILES = 196            # tiles of 128 rows per core
ROWS = TILES * 128     # 25088 ; 8*25088 = 200704 >= 200000
GROUP = 14             # tiles per pipeline group (even: B-pair matmuls)
TEMP = 30.0
EPS = 1e-6
N_ITER = 11


I32 = mybir.dt.int32


def _rsqrt_bithack(nc, pool, ss, p, n, magic, newton=2, name="rs"):
    """y ~= 1/sqrt(ss) via the int bit-hack seed + Newton refinement —
    DVE-only, so no ACT table switches (Sqrt/Ln live in different table
    sets than the softmax Exp and each switch costs ~2.7us).
    ss is clamped to >= 1e-12 so all-zero (pad) rows stay finite through
    the Newton steps (y^2 of the raw ss=0 seed would overflow f32).
    Returns a [p, n] f32 AP computing ~1/sqrt(max(ss, 1e-12))."""
    y = pool.tile([p, n], F32, name=f"{name}_y", tag=f"{name}_y")
    t1 = pool.tile([p, n], F32, name=f"{name}_t1", tag=f"{name}_t1")
    ssc = pool.tile([p, n], F32, name=f"{name}_ssc", tag=f"{name}_ssc")
    nc.vector.tensor_scalar(ssc[:], ss, 1e-12, None, ALU.max)
    yi = y.bitcast(I32)
    # int ops via tensor_tensor with pre-expanded constant tiles (no
    # immediates / no in0-broadcast: safest DVE int path on HW)
    nc.vector.tensor_tensor(yi, ssc.bitcast(I32), magic[0:p, 0:n, 1],
                            ALU.logical_shift_right)
    nc.vector.tensor_tensor(yi, magic[0:p, 0:n, 0], yi, ALU.subtract)
    for _ in range(newton):
        # y <- y * (1.5 - 0.5 * ss * y^2)
        nc.vector.tensor_mul(t1[:], y[:], y[:])
        nc.vector.tensor_mul(t1[:], ssc[:], t1[:])
        nc.vector.tensor_scalar(t1[:], t1[:], -0.5, 1.5, ALU.mult, ALU.add)
        nc.vector.tensor_mul(y[:], y[:], t1[:])
    return y


def build_cluster_kernel(tc, outs, ins, cores=N_CORES, tiles=TILES,
                         group=GROUP, n_iter=N_ITER, temp=TEMP):
    nc = tc.nc
    x_in = ins["x_in"]      # [tiles*128, 129] f32 (col 128: 1.0 valid / 0.0 pad)
    init = ins["init"]      # [K, D] f32
    ident = ins["ident"]    # [128, 128] f32 identity
    identb = ins["identb"]  # [128, 128] bf16 identity (transpose of bf16 tiles)
    fold = ins["fold64"]    # [128, 64] f32 = [I64; I64] partition-fold matrix
    r_out = outs["r_out"]   # [tiles*128, K] f32
    mu_out = outs["mu_out"]  # [K, D] f32

    n_groups = tiles // group
    assert n_groups * group == tiles and group % 2 == 0
    gk = group * K          # dist/e/r columns per group

    x_in_g = x_in.rearrange("(g t p) f -> g p t f", p=128, t=group)
    r_out_g = r_out.rearrange("(g t p) k -> g p t k", p=128, t=group)

    with tc.tile_pool(name="persist", bufs=1) as persist:
        xT_hi = persist.tile([128, tiles * 128], BF16, name="xT_hi")
        xT_lo = persist.tile([128, tiles * 128], BF16, name="xT_lo")
        x_aug = persist.tile([128, tiles, 129], BF16, name="x_aug")
        ident_sb = persist.tile([128, 128], F32, name="ident_sb")
        identb_sb = persist.tile([128, 128], BF16, name="identb_sb")
        fold_sb = persist.tile([128, K], F32, name="fold_sb")
        mu_sb = persist.tile([K, D], F32, name="mu_sb")
        magic = persist.tile([128, group, 2], I32, name="magic")

        nc.vector.memset(magic[:, :, 0:1], 0x5F3759DF)
        nc.vector.memset(magic[:, :, 1:2], 1)
        nc.sync.dma_start(ident_sb[:], ident[:])
        nc.sync.dma_start(identb_sb[:], identb[:])
        nc.sync.dma_start(fold_sb[:], fold[:])
        nc.sync.dma_start(mu_sb[:], init[:])

        # ---------------- startup: normalize rows, build splits ----------
        with tc.tile_pool(name="ld", bufs=2) as ld, \
             tc.tile_pool(name="ldw", bufs=2) as ldw, \
             tc.tile_pool(name="ldps", bufs=3, space="PSUM") as ldps:
            for g in range(n_groups):
                xin_g = ld.tile([128, group, 129], F32, name="xin_g")
                nc.sync.dma_start(xin_g[:], x_in_g[g])
                xd = xin_g[:, :, 0:128]
                # xn doubles as the Square scratch before being overwritten
                # with the normalized rows (saves SBUF).
                xn = ldw.tile([128, group, 128], F32, name="xn")
                nc.scalar.activation(xn[:], xd, AF.Square)
                ss = ldw.tile([128, group], F32, name="ss")
                nc.vector.reduce_sum(ss[:], xn[:],
                                     axis=mybir.AxisListType.X)
                # inv = 1/(sqrt(ss)+eps): rsqrt then norm=ss*y, +eps, recip
                y = _rsqrt_bithack(nc, ldw, ss[:], 128, group, magic,
                                   newton=2, name="ld")
                nrm = ldw.tile([128, group], F32, name="nrm")
                nc.vector.tensor_mul(nrm[:], ss[:], y[:])
                nc.vector.tensor_scalar(nrm[:], nrm[:], 1.0, EPS,
                                        ALU.mult, ALU.add)
                invn = ldw.tile([128, group], F32, name="invn")
                nc.vector.reciprocal(invn[:], nrm[:])
                nc.vector.tensor_tensor(
                    xn[:], xd, invn.broadcast_to([128, group, 128]), ALU.mult)
                # x_aug: bf16(xn) data cols + validity flag col
                sl = slice(g * group, (g + 1) * group)
                nc.vector.tensor_copy(x_aug[:, sl, 0:128], xn[:])
                nc.vector.tensor_copy(x_aug[:, sl, 128:129],
                                      xin_g[:, :, 128:129])
                # lo residual (bf16): xn - bf16(xn)
                xl_g = ldw.tile([128, group, 128], BF16, name="xl_g")
                nc.vector.tensor_tensor(xl_g[:], xn[:],
                                        x_aug[:, sl, 0:128], ALU.subtract)
                # transposes into xT_hi / xT_lo
                for j in range(group):
                    t = g * group + j
                    th = ldps.tile([128, 128], BF16, name="th", tag="th")
                    nc.tensor.transpose(th[:], x_aug[:, t, 0:128],
                                        identb_sb[:])
                    tl = ldps.tile([128, 128], BF16, name="tl", tag="tl")
                    nc.tensor.transpose(tl[:], xl_g[:, j, :], identb_sb[:])
                    # split the PSUM->SBUF copies between ACT and DVE
                    nc.scalar.copy(xT_hi[:, t * 128:(t + 1) * 128], th[:])
                    nc.vector.tensor_copy(xT_lo[:, t * 128:(t + 1) * 128],
                                          tl[:])

        # ---------------- iterations ------------------------------------
        with tc.tile_pool(name="wk", bufs=3) as wk, \
             tc.tile_pool(name="sbuf_s", bufs=2) as sbuf_s, \
             tc.tile_pool(name="mupool", bufs=2) as mupool, \
             tc.tile_pool(name="ps", bufs=2, space="PSUM") as ps, \
             tc.tile_pool(name="ps_cm", bufs=1, space="PSUM") as ps_cm, \
             tc.tile_pool(name="ps_t", bufs=1, space="PSUM") as ps_t, \
             tc.tile_pool(name="dram", bufs=2, space="DRAM") as dram:
            for it in range(n_iter):
                last = (it == n_iter - 1)
                # --- mun (normalized mu), bf16 hi/lo, transposed ---------
                # it==0: mun = init/||init||; else mun = cm/||cm|| (== the
                # row-normalized cm/cr since cr > 0; eps effect ~1e-6).
                src = mu_sb[:] if it == 0 else stats_ps[:, 0:128]
                musq = mupool.tile([K, D], F32, name="musq")
                mss = mupool.tile([K, 1], F32, name="mss")
                nc.scalar.activation(musq[:], src, AF.Square,
                                     accum_out=mss[:, 0:1])
                ymu = _rsqrt_bithack(nc, mupool, mss[:, 0:1], K, 1, magic,
                                      newton=3, name="mu")
                mun = mupool.tile([K, D], F32, name="mun")
                nc.vector.tensor_scalar(mun[:], src, ymu[:], None, ALU.mult)
                mh = mupool.tile([K, D], BF16, name="mh")
                nc.vector.tensor_copy(mh[:], mun[:])
                ml = mupool.tile([K, D], BF16, name="ml")
                nc.vector.tensor_tensor(ml[:], mun[:], mh[:], ALU.subtract)
                mhT = mupool.tile([128, K], BF16, name="mhT")
                mlT = mupool.tile([128, K], BF16, name="mlT")
                for msrc, mdst in ((mh, mhT), (ml, mlT)):
                    tp = ps_t.tile([128, K], BF16, name="tp", tag="tp")
                    nc.tensor.transpose(tp[:], msrc[:], identb_sb[0:K, 0:K])
                    nc.scalar.copy(mdst[:], tp[:])

                s_buf = sbuf_s.tile([128, tiles], F32, name="s_buf")
                inv_s = sbuf_s.tile([128, tiles], F32, name="inv_s")
                cm_ps = ps_cm.tile([128, 258], F32, name="cm_ps")

                for g in range(n_groups):
                    dist_ps = ps.tile([128, gk], F32, name="dist_ps")
                    for j in range(group):
                        t = g * group + j
                        dsl = dist_ps[:, j * K:(j + 1) * K]
                        xh_t = xT_hi[:, t * 128:(t + 1) * 128]
                        xl_t = xT_lo[:, t * 128:(t + 1) * 128]
                        nc.tensor.matmul(dsl, xh_t, mhT[:],
                                         start=True, stop=False)
                        nc.tensor.matmul(dsl, xh_t, mlT[:],
                                         start=False, stop=False)
                        nc.tensor.matmul(dsl, xl_t, mhT[:],
                                         start=False, stop=True)
                    e_g = wk.tile([128, gk], F32 if last else BF16, name="e_g",
                                  tag="e_g32" if last else "e_g16",
                                  bufs=2 if last else None)
                    nc.scalar.activation(e_g[:], dist_ps[:], AF.Exp, scale=temp)
                    sl = slice(g * group, (g + 1) * group)
                    nc.vector.reduce_sum(
                        s_buf[:, sl],
                        e_g.rearrange("p (t k) -> p t k", t=group),
                        axis=mybir.AxisListType.X)
                    nc.vector.reciprocal(inv_s[:, sl], s_buf[:, sl])
                    inv_b = inv_s[:, sl].broadcast_to([128, group, K])
                    r_g = wk.tile([128, group, K], BF16, name="r_g")
                    nc.vector.tensor_tensor(
                        r_g[:], e_g.rearrange("p (t k) -> p t k", t=group),
                        inv_b, ALU.mult)
                    if last:
                        r32_g = wk.tile([128, group, K], F32, name="r32_g",
                                        bufs=2)
                        nc.vector.tensor_tensor(
                            r32_g[:], e_g.rearrange("p (t k) -> p t k", t=group),
                            inv_b, ALU.mult)
                        nc.sync.dma_start(r_out_g[g], r32_g[:])
                    # cluster-mean matmuls in pairs: one 128-col weight
                    # load for two tiles; good quadrants: rows 0:64 x
                    # cols 0:129 (tile t) and rows 64:128 x cols 129:258
                    # (tile t+1).
                    for j in range(0, group, 2):
                        t = g * group + j
                        nc.tensor.matmul(
                            cm_ps[:, :],
                            r_g[:, j:j + 2, :],
                            x_aug[:, t:t + 2, :],
                            start=(t == 0), stop=(t == tiles - 2))

                # --- pre-fold good quadrants -> partial [K, 129] ---------
                cm_sb = mupool.tile([128, 258], F32, name="cm_sb")
                nc.scalar.copy(cm_sb[:], cm_ps[:])
                partial_ps = ps_t.tile([K, 129], F32, name="partial_ps",
                                       tag="foldout")
                nc.tensor.matmul(partial_ps[:], ident_sb[:, 0:K],
                                 cm_sb[:, 0:129], start=True, stop=False)
                nc.tensor.matmul(partial_ps[:], ident_sb[:, K:128],
                                 cm_sb[:, 129:258], start=False, stop=True)

                # --- AllGather + local reduce ----------------------------
                if cores > 1:
                    partial_sb = mupool.tile([K, 129], F32, name="partial_sb")
                    nc.scalar.copy(partial_sb[:], partial_ps[:])
                    ag_in = dram.tile([K, 129], F32, name="ag_in")
                    ag_out = dram.tile([K * cores, 129], F32, name="ag_out",
                                       addr_space="Shared" if cores > 4 else "Local")
                    nc.sync.dma_start(ag_in[:], partial_sb[:])
                    nc.gpsimd.collective_compute(
                        "AllGather", ALU.bypass,
                        replica_groups=[list(range(cores))],
                        ins=[ag_in.opt()], outs=[ag_out.opt()])
                    pairs = (K * cores) // 128
                    gath = mupool.tile([128, max(pairs, 1), 129], F32,
                                       name="gath")
                    nc.sync.dma_start(
                        gath[:], ag_out.rearrange("(a p) f -> p a f", p=128))
                    w = pairs
                    cur = gath
                    while w > 1:
                        w //= 2
                        nxt = mupool.tile([128, w, 129], F32,
                                          name=f"red{w}", tag=f"red{w}")
                        nc.vector.tensor_add(nxt[:], cur[:, 0:w, :],
                                             cur[:, w:2 * w, :])
                        cur = nxt
                    total = cur.rearrange("p a f -> p (a f)")
                    # stats[k,:] = total[k,:] + total[k+64,:] via fold mm
                    stats_ps = ps_t.tile([K, 129], F32, name="stats_ps",
                                         tag="foldout")
                    nc.tensor.matmul(stats_ps[:], fold_sb[:], total,
                                     start=True, stop=True)
                else:
                    stats_ps = partial_ps

                if last:
                    # mu = cluster_mean / cluster_r (not normalized)
                    crinv = mupool.tile([K, 1], F32, name="crinv")
                    nc.vector.reciprocal(crinv[:], stats_ps[:, 128:129])
                    mu_fin = mupool.tile([K, D], F32, name="mu_fin")
                    nc.vector.tensor_scalar(mu_fin[:], stats_ps[:, 0:128],
                                            crinv[:], None, ALU.mult)
                    nc.sync.dma_start(mu_out[:], mu_fin[:])


# ----------------------------------------------------------------------------
# host wrapper
# ----------------------------------------------------------------------------
_CACHED = {}


def _build_hw():
    if "nc" in _CACHED:
        return _CACHED["nc"]
    nc = bacc.Bacc("TRN2", target_bir_lowering=False, debug=False,
                   enable_asserts=False, num_devices=N_CORES)
    ins = {
        "x_in": nc.dram_tensor("x_in", [ROWS, 129], F32,
                               kind="ExternalInput").ap(),
        "init": nc.dram_tensor("init", [K, D], F32,
                               kind="ExternalInput").ap(),
        "ident": nc.dram_tensor("ident", [128, 128], F32,
                                kind="ExternalInput").ap(),
        "identb": nc.dram_tensor("identb", [128, 128], BF16,
                                 kind="ExternalInput").ap(),
        "fold64": nc.dram_tensor("fold64", [128, K], F32,
                                 kind="ExternalInput").ap(),
    }
    outs = {
        "r_out": nc.dram_tensor("r_out", [ROWS, K], F32,
                                kind="ExternalOutput").ap(),
        "mu_out": nc.dram_tensor("mu_out", [K, D], F32,
                                 kind="ExternalOutput").ap(),
    }
    with tile.TileContext(nc) as tc:
        build_cluster_kernel(tc, outs, ins)
    nc.compile()
    _CACHED["nc"] = nc
    return nc


def _prep_in_maps(embeds, init):
    n = embeds.shape[0]
    x = np.zeros((ROWS * N_CORES, 129), dtype=np.float32)
    x[:n, 0:128] = embeds
    x[:n, 128] = 1.0
    import ml_dtypes
    ident = np.eye(128, dtype=np.float32)
    identb = np.eye(128, dtype=np.float32).astype(ml_dtypes.bfloat16)
    fold64 = np.vstack([np.eye(K, dtype=np.float32),
                        np.eye(K, dtype=np.float32)])
    init = np.ascontiguousarray(np.asarray(init, np.float32))
    return [
        {"x_in": np.ascontiguousarray(x[c * ROWS:(c + 1) * ROWS]),
         "init": init, "ident": ident, "identb": identb, "fold64": fold64}
        for c in range(N_CORES)
    ]


def run_hw(embeds, init, trace=False):
    nc = _build_hw()
    in_maps = _prep_in_maps(embeds, init)
    res = run_bass_kernel_spmd(nc, in_maps,
                               core_ids=list(range(N_CORES)), trace=trace)
    r = np.concatenate([res.results[c]["r_out"] for c in range(N_CORES)],
                       axis=0)[:N_FULL]
    mu = res.results[0]["mu_out"]
    return (mu, r), res


def kernel(embeds, init, cluster_temp):
    assert int(np.asarray(cluster_temp)) == 30
    (mu, r), _ = run_hw(np.asarray(embeds, np.float32),
                        np.asarray(init, np.float32))
    return mu, r
